# revision 1
# baseline (speedup 1.0000x reference)
"""Bass/Trainium2 kernel for the EvolutionAgentQuantum circuit.

10-qubit state-vector simulation, batch 4096, 5 layers of
[per-sample RY] -> [fused RZ diagonal] -> [shared RY] -> [CNOT ring],
then 4 Pauli-Z expectations. Data-parallel over 8 NeuronCores
(512 samples each), 4 partition-chains of 128 samples per core.

Layout per chain: state re/im merged in one SBUF tile [128, 2048]
(free = comp*1024 + amplitude; qubit q has amplitude stride 2^(9-q)).

Algebra:
 - RY(theta) = cos(t) * [[1, -tan(t)], [tan(t), 1]], t = theta/2.
   Butterflies apply only the tan part (2 fused scalar_tensor_tensor
   ops per gate); all cos factors of a layer are folded into one
   per-layer per-sample rescale (applied to the 4 outputs as C^2
   via the squared amplitudes).
 - All 20 RZ gates of a layer commute into one diagonal
   exp(i * phase[s]), phase[s] = sum_i (+-) phi_i/2, built by a
   doubling recursion on ScalarE, range-reduced to [-pi, pi]
   (magic-number round + Cody-Waite), then Sin activations.
"""

import sys
import os

for _p in ("/opt/trn_rl_repo", "/root/.axon_site/_ro/trn_rl_repo"):
    if os.path.isdir(_p) and _p not in sys.path:
        sys.path.insert(0, _p)

import numpy as np

import concourse.bacc as bacc
from concourse import mybir
from concourse.bass_utils import run_bass_kernel_spmd
from concourse.tile import TileContext

AF = mybir.ActivationFunctionType
ALU = mybir.AluOpType
F32 = mybir.dt.float32
F32R = mybir.dt.float32r

N_CORES = 8
BATCH = 4096
OBS = 10
NQ = 10
NL = 5
NOUT = 4
PER_CORE = BATCH // N_CORES      # 512
P = 128
NCHAIN = PER_CORE // P           # 4
NAMP = 1 << NQ                   # 1024
SFREE = 2 * NAMP                 # 2048 (re | im)

PI = float(np.pi)
HALF_PI = float(np.pi / 2)
TWO_PI = float(2 * np.pi)
MAGIC = float(1.5 * 2 ** 23)
# Cody-Waite 3-term split of 2*pi
CW1 = 6.28125
CW2 = float(np.float32(2 * np.pi - CW1))
CW3 = float(2 * np.pi - CW1 - np.float32(2 * np.pi - CW1))

# host table layout (columns of the broadcast [128, TBL_LEN] tile)
O_YS = 0      # 50: theta_y/2 scale  (0.5*isc[l, i])
O_ZS = 50     # 50: phi_x/2 scale    (0.5*isc[l, i+10])
O_ZW = 100    # 50: phi_w/2 additive (0.5*w[l, i])
O_TW = 150    # 50: tan(w[l, i+10]/2)
O_NTW = 200   # 50: -tan(w[l, i+10]/2)
O_CWP = 250   # 5:  prod_i cos(w[l, i+10]/2)
O_OS = 255    # 4:  output_scaling
TBL_LEN = 259

_CACHED_NC = {}


def _emit_butterfly(nc, src, dst, q, tcol, ntcol):
    """dst = un-normalized RY on qubit q of src (both [128, SFREE] tiles).

    u0 = s0 - t*s1 ; u1 = t*s0 + s1  (cos factor deferred).
    Applied to re and im at once via the comp-merged view.
    """
    co = 1 << (q + 1)
    inner = 1 << (9 - q)
    sv = src[:].rearrange("p (co t i) -> p co t i", co=co, t=2, i=inner)
    dv = dst[:].rearrange("p (co t i) -> p co t i", co=co, t=2, i=inner)
    s0 = sv[:, :, 0, :]
    s1 = sv[:, :, 1, :]
    nc.vector.scalar_tensor_tensor(
        dv[:, :, 0, :], s1, ntcol, s0, ALU.mult, ALU.add
    )
    nc.vector.scalar_tensor_tensor(
        dv[:, :, 1, :], s0, tcol, s1, ALU.mult, ALU.add
    )


def _emit_cnot(nc, state, ctmp, c, t):
    """In-place CNOT(control=c, target=t) on state [128, SFREE].

    Swaps the two target-halves of the control=1 slab (3 ScalarE copies
    through ctmp [128, 512])."""
    if c < t:
        co = 1 << (c + 1)            # comp + bits above c, stride 2^(10-c)
        m = 1 << (t - c - 1)
        inner = 1 << (9 - t)
        v = state[:].rearrange(
            "p (co a m b i) -> p co a m b i", co=co, a=2, m=m, b=2, i=inner
        )
        A = v[:, :, 1, :, 0, :]
        B = v[:, :, 1, :, 1, :]
        tv = ctmp[:].rearrange("p (co m i) -> p co m i", co=co, m=m, i=inner)
    else:
        # the ring-closing CNOT(9, 0): control = LSB, target = MSB
        assert c == 9 and t == 0
        v = state[:].rearrange(
            "p (c b m a) -> p c b m a", c=2, b=2, m=256, a=2
        )
        A = v[:, :, 0, :, 1:2]
        B = v[:, :, 1, :, 1:2]
        tv = ctmp[:].rearrange("p (c m i) -> p c m i", c=2, m=256, i=1)
    nc.scalar.copy(tv, A)
    nc.scalar.copy(A, B)
    nc.scalar.copy(B, tv)


def _build_nc(rep=1):
    nc = bacc.Bacc()
    x_d = nc.declare_dram_parameter("x", [PER_CORE, OBS], F32, isOutput=False)
    tbl_d = nc.declare_dram_parameter("tbl", [TBL_LEN], F32, isOutput=False)
    wm_d = nc.declare_dram_parameter("wm", [NL, NAMP, NAMP], F32R, isOutput=False)
    id_d = nc.declare_dram_parameter("ident", [P, P], F32, isOutput=False)
    out_d = nc.declare_dram_parameter("out", [PER_CORE, NOUT], F32, isOutput=True)

    with TileContext(nc) as tc:
        with tc.tile_pool(name="pool", bufs=1) as pool, \
             tc.tile_pool(name="psum", bufs=4, space="PSUM") as psum:
            # shared constants
            tbl = pool.tile([P, TBL_LEN], F32, tag="tbl")
            nc.sync.dma_start(
                out=tbl[:], in_=tbl_d[:].unsqueeze(0).to_broadcast((P, TBL_LEN))
            )
            ident = pool.tile([P, P], F32, tag="ident")
            nc.sync.dma_start(out=ident[:], in_=id_d[:])
            wt_a = pool.tile([P, 4 * NAMP], F32R, tag="wt_a")
            wt_b = pool.tile([P, 4 * NAMP], F32R, tag="wt_b")
            stb2 = [pool.tile([P, SFREE], F32R, tag="stb0", name="stb0"),
                    pool.tile([P, SFREE], F32R, tag="stb1", name="stb1")]
            c_halfpi = pool.tile([P, 1], F32, tag="c_halfpi")
            c_inv2pi = pool.tile([P, 1], F32, tag="c_inv2pi")
            c_magic = pool.tile([P, 1], F32, tag="c_magic")
            c_nmagic = pool.tile([P, 1], F32, tag="c_nmagic")
            c_none = pool.tile([P, 1], F32, tag="c_none")
            nc.vector.memset(c_halfpi[:], HALF_PI)
            nc.vector.memset(c_inv2pi[:], 1.0 / TWO_PI)
            nc.vector.memset(c_magic[:], MAGIC)
            nc.vector.memset(c_nmagic[:], -MAGIC)
            nc.vector.memset(c_none[:], -1.0)

            # ---- per-chain setup: x DMA + angle tables ----
            C = [dict() for _ in range(NCHAIN)]
            for ch in range(NCHAIN):
                tg = f"_{ch}"
                d = C[ch]
                xt = pool.tile([P, OBS], F32, tag="xt" + tg)
                nc.sync.dma_start(out=xt[:], in_=x_d[ch * P : (ch + 1) * P, :])

                ty = pool.tile([P, 5 * OBS], F32, tag="ty" + tg)
                sy = pool.tile([P, 5 * OBS], F32, tag="sy" + tg)
                cy = pool.tile([P, 5 * OBS], F32, tag="cy" + tg)
                rcy = pool.tile([P, 5 * OBS], F32, tag="rcy" + tg)
                nty = pool.tile([P, 5 * OBS], F32, tag="nty" + tg)
                ph = pool.tile([P, 5 * OBS], F32, tag="ph" + tg)
                nph = pool.tile([P, 5 * OBS], F32, tag="nph" + tg)
                clp = pool.tile([P, NL], F32, tag="clp" + tg)
                csc = pool.tile([P, NL], F32, tag="csc" + tg)

                xb = xt[:].unsqueeze(1).to_broadcast((P, NL, OBS))
                tyv = ty[:].rearrange("p (l q) -> p l q", l=NL)
                ysv = tbl[:, O_YS : O_YS + 50].rearrange("p (l q) -> p l q", l=NL)
                nc.vector.tensor_tensor(tyv, xb, ysv, ALU.mult)
                nc.scalar.activation(sy[:], ty[:], AF.Sin)
                nc.vector.add_range_wrap(cy[:], ty[:], HALF_PI, PI, TWO_PI)
                nc.scalar.activation(cy[:], cy[:], AF.Sin)
                nc.vector.reciprocal(rcy[:], cy[:])
                nc.vector.tensor_tensor(ty[:], sy[:], rcy[:], ALU.mult)
                nc.vector.tensor_scalar_mul(nty[:], ty[:], -1.0)
                for l in range(NL):
                    nc.vector.tensor_reduce(
                        clp[:, l : l + 1], cy[:, 10 * l : 10 * l + 10],
                        mybir.AxisListType.X, ALU.mult,
                    )
                phv = ph[:].rearrange("p (l q) -> p l q", l=NL)
                zsv = tbl[:, O_ZS : O_ZS + 50].rearrange("p (l q) -> p l q", l=NL)
                nc.vector.tensor_tensor(phv, xb, zsv, ALU.mult)
                nc.vector.tensor_tensor(
                    ph[:], ph[:], tbl[:, O_ZW : O_ZW + 50], ALU.add
                )
                nc.vector.tensor_scalar_mul(nph[:], ph[:], -1.0)

                d["ty"], d["nty"], d["ph"], d["nph"] = ty, nty, ph, nph
                d["clp"] = clp
                d["sa"] = pool.tile([P, SFREE], F32, tag="sa" + tg, name="sa" + tg)
                d["sb"] = pool.tile([P, SFREE], F32, tag="sb" + tg, name="sb" + tg)
                d["pht"] = pool.tile([P, NAMP], F32, tag="pht" + tg, name="pht" + tg)
                d["cosT"] = pool.tile([P, NAMP], F32, tag="cosT" + tg, name="cosT" + tg)
                d["sinT"] = pool.tile([P, NAMP], F32, tag="sinT" + tg, name="sinT" + tg)
                d["tmpA"] = pool.tile([P, NAMP], F32, tag="tmpA" + tg, name="tmpA" + tg)
                d["cur"], d["oth"] = d["sa"], d["sb"]

            def col(t, l, i):
                return t[:, 10 * l + i : 10 * l + i + 1]

            # ---- circuit, layer-major (W tile shared across chains) ----
            from contextlib import nullcontext
            loop_cm = tc.For_i(0, rep, 1) if rep > 1 else nullcontext()
            with loop_cm:
              for _rep in range(1):
                for l in range(NL):
                    # load this layer's 1024x1024 weight matrix (lhsT blocks)
                    nc.sync.dma_start(
                        out=wt_a[:].rearrange("p (r m) -> p r m", r=4),
                        in_=wm_d[l, 0 : 4 * P].rearrange("(r p) m -> p r m", p=P),
                    )
                    nc.sync.dma_start(
                        out=wt_b[:].rearrange("p (r m) -> p r m", r=4),
                        in_=wm_d[l, 4 * P : 8 * P].rearrange("(r p) m -> p r m", p=P),
                    )
                    for ch in range(NCHAIN):
                        d = C[ch]
                        stb = stb2[ch % 2]
                        cur, oth = d["cur"], d["oth"]
                        ty, nty, ph, nph = d["ty"], d["nty"], d["ph"], d["nph"]
                        pht, cosT, sinT, tmpA = (
                            d["pht"], d["cosT"], d["sinT"], d["tmpA"]
                        )

                        # ---- per-sample RY layer ----
                        if l == 0 and _rep == 0:
                            nc.vector.memset(cur[:, 0:1], 1.0)
                            for j in range(9, -1, -1):
                                g = 1 << (9 - j)
                                nc.vector.tensor_scalar_mul(
                                    cur[:, g : 2 * g], cur[:, 0:g], col(ty, 0, j)
                                )
                        elif l == 0:
                            # rep>0 timing loops: rebuild as in l==0
                            nc.vector.memset(cur[:, 0:1], 1.0)
                            for j in range(9, -1, -1):
                                g = 1 << (9 - j)
                                nc.vector.tensor_scalar_mul(
                                    cur[:, g : 2 * g], cur[:, 0:g], col(ty, 0, j)
                                )
                        else:
                            for i in range(NQ):
                                _emit_butterfly(
                                    nc, cur, oth, i, col(ty, l, i), col(nty, l, i)
                                )
                                cur, oth = oth, cur

                        # ---- fused RZ diagonal ----
                        nc.scalar.activation(
                            pht[:, 0:1], col(ph, l, 9), AF.Identity,
                            scale=c_none[:],
                        )
                        nc.scalar.copy(pht[:, 1:2], col(ph, l, 9))
                        for j in range(8, -1, -1):
                            g = 1 << (9 - j)
                            nc.scalar.activation(
                                pht[:, g : 2 * g], pht[:, 0:g], AF.Identity,
                                bias=col(ph, l, j),
                            )
                            nc.scalar.activation(
                                pht[:, 0:g], pht[:, 0:g], AF.Identity,
                                bias=col(nph, l, j),
                            )
                        nc.scalar.activation(
                            tmpA[:], pht[:], AF.Identity,
                            scale=c_inv2pi[:], bias=c_magic[:],
                        )
                        nc.scalar.activation(
                            tmpA[:], tmpA[:], AF.Identity, bias=c_nmagic[:]
                        )
                        nc.vector.cody_waite_cascade(
                            cosT[:], pht[:], tmpA[:], CW1, CW2, CW3
                        )
                        nc.scalar.activation(sinT[:], cosT[:], AF.Sin)
                        nc.vector.add_range_wrap(
                            tmpA[:], cosT[:], HALF_PI, PI, TWO_PI
                        )
                        nc.scalar.activation(cosT[:], tmpA[:], AF.Sin)

                        sre = cur[:, 0:NAMP]
                        sim = cur[:, NAMP:SFREE]
                        dre = oth[:, 0:NAMP]
                        dim = oth[:, NAMP:SFREE]
                        if l == 0:
                            nc.gpsimd.tensor_tensor(dre, sre, cosT[:], ALU.mult)
                            nc.gpsimd.tensor_tensor(dim, sre, sinT[:], ALU.mult)
                        else:
                            nc.gpsimd.tensor_tensor(dre, sre, cosT[:], ALU.mult)
                            nc.gpsimd.tensor_tensor(tmpA[:], sim, sinT[:], ALU.mult)
                            nc.gpsimd.tensor_tensor(dre, dre, tmpA[:], ALU.subtract)
                            nc.gpsimd.tensor_tensor(dim, sre, sinT[:], ALU.mult)
                            nc.gpsimd.tensor_tensor(tmpA[:], sim, cosT[:], ALU.mult)
                            nc.gpsimd.tensor_tensor(dim, dim, tmpA[:], ALU.add)
                        cur, oth = oth, cur

                        # ---- shared RY + CNOT ring as one matmul on PE ----
                        # state-stationary form: out_A[samp, amp'] =
                        #   sum_amp stB[amp, samp] * W[amp, amp'], so the
                        #   result lands directly back in layout A (no back
                        #   transposes). stB blocks (lhsT) live in `oth`.
                        for comp in range(2):
                            for rg in range(2):
                                pt = psum.tile([P, 4 * P], F32, tag="ps_t", name="ps_t")
                                for rr in range(4):
                                    r = rg * 4 + rr
                                    nc.tensor.transpose(
                                        pt[:, rr * P : (rr + 1) * P],
                                        cur[:, comp * NAMP + r * P : comp * NAMP + (r + 1) * P],
                                        ident[:],
                                    )
                                nc.scalar.copy(
                                    stb[:, comp * NAMP + rg * 4 * P : comp * NAMP + (rg + 1) * 4 * P],
                                    pt[:],
                                )
                        for comp in range(2):
                            pm0 = psum.tile([P, 4 * P], F32, tag="ps_mm", name="ps_mm")
                            pm1 = psum.tile([P, 4 * P], F32, tag="ps_mm", name="ps_mm")
                            for r in range(8):
                                lhsT = stb[:, comp * NAMP + r * P : comp * NAMP + (r + 1) * P]
                                wh = wt_a if r < 4 else wt_b
                                rr = r % 4
                                nc.tensor.matmul(
                                    pm0[:], lhsT,
                                    wh[:, rr * NAMP : rr * NAMP + 4 * P],
                                    start=(r == 0), stop=(r == 7),
                                )
                                nc.tensor.matmul(
                                    pm1[:], lhsT,
                                    wh[:, rr * NAMP + 4 * P : (rr + 1) * NAMP],
                                    start=(r == 0), stop=(r == 7),
                                )
                            nc.scalar.activation(
                                cur[:, comp * NAMP : comp * NAMP + 4 * P],
                                pm0[:], AF.Identity,
                                scale=d["clp"][:, l : l + 1],
                            )
                            nc.scalar.activation(
                                cur[:, comp * NAMP + 4 * P : (comp + 1) * NAMP],
                                pm1[:], AF.Identity,
                                scale=d["clp"][:, l : l + 1],
                            )

                        d["cur"], d["oth"] = cur, oth

            # ---- observables ----
            for ch in range(NCHAIN):
                tg = f"_{ch}"
                d = C[ch]
                cur = d["cur"]
                pht, tmpA = d["pht"], d["tmpA"]
                zt = pool.tile([P, 8], F32, tag="zt" + tg)
                ot = pool.tile([P, NOUT], F32, tag="ot" + tg)
                sre = cur[:, 0:NAMP]
                sim = cur[:, NAMP:SFREE]
                nc.gpsimd.tensor_tensor(pht[:], sre, sre, ALU.mult)
                nc.gpsimd.tensor_tensor(tmpA[:], sim, sim, ALU.mult)
                nc.gpsimd.tensor_tensor(pht[:], pht[:], tmpA[:], ALU.add)
                nc.vector.tensor_reduce(
                    zt[:, 4:5], pht[:], mybir.AxisListType.X, ALU.add
                )
                for i in range(NOUT):
                    o = 1 << i
                    inner = 1 << (9 - i)
                    pv = pht[:].rearrange("p (o t i) -> p o t i", o=o, t=2, i=inner)
                    nc.vector.tensor_reduce(
                        zt[:, i : i + 1], pv[:, :, 0, :],
                        mybir.AxisListType.XY, ALU.add,
                    )
                for i in range(NOUT):
                    nc.vector.tensor_scalar(
                        ot[:, i : i + 1], zt[:, i : i + 1], 2.0, zt[:, 4:5],
                        ALU.mult, ALU.subtract,
                    )
                nc.vector.tensor_tensor(
                    ot[:], ot[:], tbl[:, O_OS : O_OS + NOUT], ALU.mult
                )
                nc.sync.dma_start(
                    out=out_d[ch * P : (ch + 1) * P, :], in_=ot[:]
                )

    nc.compile()
    return nc


def _host_table(input_scaling, weights, output_scaling):
    isc = np.asarray(input_scaling, np.float64)
    w = np.asarray(weights, np.float64)
    os_ = np.asarray(output_scaling, np.float64)
    ys = 0.5 * isc[:, :OBS]
    zs = 0.5 * isc[:, OBS : 2 * OBS]
    zw = 0.5 * w[:, :NQ]
    a = 0.5 * w[:, NQ : 2 * NQ]
    tw = np.tan(a)
    cwp = np.prod(np.cos(a), axis=1)
    tbl = np.concatenate(
        [ys.ravel(), zs.ravel(), zw.ravel(), tw.ravel(), (-tw).ravel(), cwp, os_]
    ).astype(np.float32)
    assert tbl.shape[0] == TBL_LEN
    return tbl


def _host_mats(weights):
    """Per-layer 1024x1024 lhsT matrices: W_l = (P_ring @ kron_i RY(w2_i)).T"""
    w = np.asarray(weights, np.float64)
    # CNOT-ring permutation L: bits b0(MSB)..b9; b_{i+1}^=b_i (i=0..8), b0^=b9
    s = np.arange(NAMP)
    bits = [(s >> (9 - j)) & 1 for j in range(10)]
    for i in range(9):
        bits[i + 1] = bits[i + 1] ^ bits[i]
    bits[0] = bits[0] ^ bits[9]
    L = np.zeros(NAMP, np.int64)
    for j in range(10):
        L |= bits[j] << (9 - j)
    wm = np.empty((NL, NAMP, NAMP), np.float32)
    for l in range(NL):
        M = np.array([[1.0]])
        for i in range(NQ):
            a = 0.5 * w[l, NQ + i]
            c, sn = np.cos(a), np.sin(a)
            M = np.kron(M, np.array([[c, -sn], [sn, c]]))
        Ml = np.zeros_like(M)
        Ml[L, :] = M          # ring permutation applied after the rotations
        wm[l] = Ml.T.astype(np.float32)
    return wm


def kernel(x, input_scaling, weights, output_scaling):
    global _CACHED_NC
    x = np.ascontiguousarray(np.asarray(x, np.float32))
    tbl = _host_table(input_scaling, weights, output_scaling)
    wm = _host_mats(weights)
    ident = np.eye(P, dtype=np.float32)

    if 1 not in _CACHED_NC:
        _CACHED_NC[1] = _build_nc(1)
    nc = _CACHED_NC[1]

    in_maps = [
        {"x": x[c * PER_CORE : (c + 1) * PER_CORE], "tbl": tbl,
         "wm": wm, "ident": ident}
        for c in range(N_CORES)
    ]
    res = run_bass_kernel_spmd(nc, in_maps, list(range(N_CORES))).results
    return np.concatenate([r["out"] for r in res], axis=0)


if __name__ == "__main__":
    rng = np.random.default_rng(0)
    x = rng.standard_normal((BATCH, OBS)).astype(np.float32)
    isc = np.ones((NL, 2 * NQ), np.float32)
    w = rng.uniform(-np.pi, np.pi, (NL, 2 * NQ)).astype(np.float32)
    os_ = np.ones((NOUT,), np.float32)
    out = kernel(x, isc, w, os_)
    print(out.shape, out[:2])

